# revision 9
# baseline (speedup 1.0000x reference)
"""Trainium2 Bass kernel for batched multi-head attention.

Problem: q, k, v: [B=4, H=16, D=64, N=2048] fp32, layout (b, h, d, n).
    sim  = einsum('bhdi,bhdj->bhij', q * D**-0.5, k)
    attn = softmax(sim, axis=-1)
    out  = einsum('bhij,bhdj->bhdi', attn, v)

Sharding: the 64 (b, h) pairs split across 8 NeuronCores, 8 heads per core
(4 pairs of 2 heads stacked on 128 partitions). No collectives.

Per-core algorithm (flash-style, no max subtraction; logits are O(1)):
  stream of 512 units (pair, sub-head, i-tile of 512, j-chunk of 128),
  grouped 2 units per 2-bank PSUM slot:
    S^T[j, i] = K_chunk^T Q_tile          (fp32r matmul, 512 rows)
    expS      = exp(S * scale) -> bf16    (split across THREE engines, see below)
    O^T[i, d|sum] += expS_chunk^T [V^T|1] (bf16 matmuls, transposed-PV: output
                                           free dim is 65, 4 matmuls per unit,
                                           accumulated over the 16 j-chunks)
    out^T[i, d] = O^T[d] / O^T[sum]       (Pool engine divide)
  V^T arrives pre-transposed from the host with a fused ones column; the
  output is written as out^T [h, n, d] tiles and un-transposed on the host.

Engine split of the 256 exp groups (the softmax exp is the scalar-engine
roofline; PE is the overall roofline at ~170us/core):
  ACT:  exact exp activation, bf16 out                     (~140 groups)
  DVE:  two-seed Schraudolph: z1 = int16(S*A + B2);        (~72 groups)
        z2 = z1 + 64; ex = bf16(z1) + bf16(z2)
        (the +64 int16 offset is a half-period shift = *sqrt(2); summing the
        two seeds phase-averages the Schraudolph sawtooth; the 1+sqrt(2)
        scale is folded into B2 and cancels in softmax normalization)
  Pool: single-seed Schraudolph z1 = int16(S*A + B1), bitcast bf16
        (~44 groups; Pool also runs the normalization divides)
"""

import numpy as np
import ml_dtypes

import concourse.bass as bass
import concourse.mybir as mybir
import concourse.tile as tile
from concourse import bacc
from concourse.bass_utils import run_bass_kernel_spmd

B, H, D, N = 4, 16, 64, 2048
NCORES = 8
HPC = (B * H) // NCORES  # heads per core = 8
NPAIRS = HPC // 2        # head pairs per core = 4
ITILE = 512              # query columns per i-tile
NIT = N // ITILE         # 4 i-tiles
JC = 128                 # key chunk (contraction partition dim)
NCH = N // JC            # 16 chunks
SCALE = float(D) ** -0.5
GW = 2                   # units per group (2-bank PSUM slots)
NGROUPS = (NPAIRS * 2 * NIT * NCH) // GW  # 256

# Schraudolph constants (bf16/int16 bit trick), tuned numerically against the
# exact softmax: z = S * A + B, truncating fp32->int16 convert.
LOG2E = 1.4426950408889634
EXP_A = float(SCALE * LOG2E * 128.0)
EXP_B2 = float(127 * 128 - np.log2(1.0 + np.sqrt(2.0)) * 128.0 - 7.0)  # two-seed
EXP_B1 = float(127 * 128 - 7.0)                                        # one-seed

# exp engine assignment: counts over the 256 groups
N_DVE1 = 77   # single-seed Schraudolph on DVE
N_DVE2 = 34   # two-seed Schraudolph on DVE (phase-averaged, ~3x lower rms)
RAMP_ACT = 6  # first groups forced to ACT (warm start)
TAIL_ACT = 4  # last groups forced to ACT (fast drain)
O_LAG = 3     # groups between exp(g) and its PV consumption O(g)

F32 = mybir.dt.float32
F32R = mybir.dt.float32r
BF16 = mybir.dt.bfloat16
I16 = mybir.dt.int16

_CACHE = {}


def assign_engines():
    """Per-group engine: 'A' (ACT exact exp), 'D' (DVE 1-seed Schraudolph),
    'E' (DVE 2-seed Schraudolph)."""
    eng = ["A"] * NGROUPS
    free = list(range(RAMP_ACT, NGROUPS - TAIL_ACT))
    nfree = len(free)
    acc_d = acc_e = 0.0
    for g in free:
        acc_d += N_DVE1 / nfree
        acc_e += N_DVE2 / nfree
        if acc_e >= 1.0:
            eng[g] = "E"
            acc_e -= 1.0
        elif acc_d >= 1.0:
            eng[g] = "D"
            acc_d -= 1.0
    return eng


def build_bass():
    nc = bacc.Bacc("TRN2", target_bir_lowering=False)
    qk_h = nc.dram_tensor("qk", [NPAIRS, 2, 128, N], F32, kind="ExternalInput")
    vt_h = nc.dram_tensor("vt", [NPAIRS, 128, 2 * NCH * 65], BF16, kind="ExternalInput")
    o_h = nc.dram_tensor("out", [HPC, NIT, 128, 4 * 64], F32, kind="ExternalOutput")

    qk_d = qk_h[:, :, :, :].rearrange("p t a n -> p a t n")  # [NPAIRS, 128, 2, N]

    eng = assign_engines()
    units = [
        (p, sub, it, c)
        for p in range(NPAIRS)
        for sub in range(2)
        for it in range(NIT)
        for c in range(NCH)
    ]
    groups = [units[i : i + GW] for i in range(0, len(units), GW)]
    assert len(groups) == NGROUPS

    with tile.TileContext(nc) as tc:
        with (
            tc.tile_pool(name="consts", bufs=1) as consts,
            tc.tile_pool(name="pairs", bufs=2) as pairs,
            tc.tile_pool(name="vtp", bufs=2) as vtp,
            tc.tile_pool(name="spsum", bufs=3, space="PSUM") as spsum,
            tc.tile_pool(name="opsum", bufs=2, space="PSUM") as opsum,
            tc.tile_pool(name="expp", bufs=6) as expp,
            tc.tile_pool(name="zp", bufs=6) as zp,
            tc.tile_pool(name="outp", bufs=4) as outp,
        ):
            # dummy exp so the ACT table load lands at t=0 on real hw
            dum = consts.tile([1, 8], F32, tag="dum")
            nc.vector.memset(dum, 0.0)
            nc.scalar.activation(out=dum, in_=dum, func=mybir.ActivationFunctionType.Exp)

            pair_ctx: list[dict | None] = [None] * NPAIRS

            def emit_pair_dma(p):
                qk = pairs.tile([128, 2, N], F32R, tag="qk", name=f"qk_{p}")
                src = qk_d[p].bitcast(F32R)
                vt = vtp.tile([128, 2, NCH, 65], BF16, tag="vt", name=f"vt_{p}")
                if p == 0:
                    # peel in dependency order: the first S group needs only
                    # q i-tile0 + the first K chunks; vt must land before O(0)
                    nc.sync.dma_start(out=qk[:, 0, 0:ITILE], in_=src[:, 0, 0:ITILE])
                    nc.sync.dma_start(out=qk[:, 1, 0:256], in_=src[:, 1, 0:256])
                    nc.sync.dma_start(out=qk[:, 1, 256:1024], in_=src[:, 1, 256:1024])
                    nc.sync.dma_start(
                        out=vt.rearrange("a b c d -> a (b c d)"), in_=vt_h[p]
                    )
                    nc.sync.dma_start(out=qk[:, 1, 1024:N], in_=src[:, 1, 1024:N])
                    nc.sync.dma_start(out=qk[:, 0, ITILE:1024], in_=src[:, 0, ITILE:1024])
                    nc.sync.dma_start(out=qk[:, 0, 1024:N], in_=src[:, 0, 1024:N])
                else:
                    nc.sync.dma_start(out=qk[:, 0:1, :], in_=src[:, 0:1, :])
                    nc.sync.dma_start(out=qk[:, 1:2, :], in_=src[:, 1:2, :])
                    nc.sync.dma_start(
                        out=vt.rearrange("a b c d -> a (b c d)"), in_=vt_h[p]
                    )
                pair_ctx[p] = {"qk": qk, "vt": vt}

            slots: dict[int, bass.AP] = {}
            exs: dict[int, bass.AP] = {}
            o_accs: dict[tuple, bass.AP] = {}
            pending_norms: list[tuple] = []

            def emit_S(g):
                slot = spsum.tile([128, GW * ITILE], F32, tag="s", name=f"s_{g}")
                for kk, (p, sub, it, c) in enumerate(groups[g]):
                    if sub == 0 and it == 0 and c == 0:
                        if p == 0:
                            emit_pair_dma(0)
                        if p + 1 < NPAIRS:
                            emit_pair_dma(p + 1)
                    qk = pair_ctx[p]["qk"]
                    hb = sub * D
                    nc.tensor.matmul(
                        out=slot[:, kk * ITILE : (kk + 1) * ITILE],
                        lhsT=qk[hb : hb + D, 1, c * JC : (c + 1) * JC],
                        rhs=qk[hb : hb + D, 0, it * ITILE : (it + 1) * ITILE],
                        start=True,
                        stop=True,
                    )
                slots[g] = slot

            def emit_X(g):
                slot = slots.pop(g)
                w = GW * ITILE
                e = eng[g]
                if e == "A":
                    ex = expp.tile([128, w], BF16, tag="exA", name=f"xa_{g}")
                    nc.scalar.activation(
                        out=ex, in_=slot, func=mybir.ActivationFunctionType.Exp,
                        scale=SCALE,
                    )
                elif e == "D":  # DVE single-seed Schraudolph
                    z1 = zp.tile([128, w], I16, tag="z1", name=f"z1_{g}")
                    nc.vector.tensor_scalar(
                        out=z1, in0=slot, scalar1=EXP_A, scalar2=EXP_B1,
                        op0=mybir.AluOpType.mult, op1=mybir.AluOpType.add,
                    )
                    ex = z1.bitcast(BF16)
                else:  # DVE two-seed Schraudolph (phase-averaged)
                    z1 = zp.tile([128, w], I16, tag="z1", name=f"z1_{g}")
                    nc.vector.tensor_scalar(
                        out=z1, in0=slot, scalar1=EXP_A, scalar2=EXP_B2,
                        op0=mybir.AluOpType.mult, op1=mybir.AluOpType.add,
                    )
                    z2 = zp.tile([128, w], I16, tag="z2", name=f"z2_{g}")
                    nc.vector.tensor_scalar(
                        out=z2, in0=z1, scalar1=64, scalar2=None,
                        op0=mybir.AluOpType.add,
                    )
                    ex = expp.tile([128, w], BF16, tag="exE", name=f"xe_{g}")
                    nc.vector.tensor_tensor(
                        out=ex, in0=z1.bitcast(BF16), in1=z2.bitcast(BF16),
                        op=mybir.AluOpType.add,
                    )
                exs[g] = ex

            def emit_O(g):
                ex = exs.pop(g)
                for kk, (p, sub, it, c) in enumerate(groups[g]):
                    vt = pair_ctx[p]["vt"]
                    if c == 0:
                        o_accs[(p, sub, it)] = opsum.tile(
                            [128, 4 * 65], F32, tag="o", name=f"oa_{g}_{kk}"
                        )
                    acc = o_accs[(p, sub, it)]
                    for si in range(4):
                        # start=True zeroes the WHOLE psum bank, so only the
                        # very first matmul of this accumulator may set it;
                        # the other si chains accumulate onto the zeroed bank.
                        nc.tensor.matmul(
                            out=acc[:, si * 65 : (si + 1) * 65],
                            lhsT=ex[:, kk * ITILE + si * 128 : kk * ITILE + (si + 1) * 128],
                            rhs=vt[:, sub, c, :],
                            start=(c == 0 and si == 0),
                            stop=(c == NCH - 1),
                        )
                    if c == NCH - 1:
                        pending_norms.append((p, sub, it))

            def emit_norm(p, sub, it):
                # Pool (gpsimd) cannot read PSUM: DVE copies the accumulator
                # to SBUF in one op, Pool does the four divides from there.
                acc = o_accs.pop((p, sub, it))
                acc_sb = outp.tile([128, 4 * 65], F32, tag="accsb", name=f"ac_{p}_{sub}_{it}")
                nc.scalar.activation(
                    out=acc_sb, in_=acc, func=mybir.ActivationFunctionType.Copy
                )
                out_sb = outp.tile([128, 4, 64], F32, tag="osb", name=f"ot_{p}_{sub}_{it}")
                for si in range(4):
                    nc.gpsimd.normalize_recip(
                        out_sb[:, si, :],
                        acc_sb[:, si * 65 : si * 65 + 64],
                        acc_sb[:, si * 65 + 64 : si * 65 + 65],
                    )
                nc.sync.dma_start(
                    out=o_h[2 * p + sub, it],
                    in_=out_sb.rearrange("a b c -> a (b c)"),
                )

            def flush_norms(g):
                while pending_norms:
                    emit_norm(*pending_norms.pop(0))

            # software-pipelined emission: PE stream ... S(g+1) O(g-1) S(g+2) ...
            emit_S(0)
            for g in range(NGROUPS):
                emit_X(g)
                if g + 1 < NGROUPS:
                    emit_S(g + 1)
                if g >= O_LAG:
                    emit_O(g - O_LAG)
                flush_norms(g)
            for g in range(NGROUPS - O_LAG, NGROUPS):
                emit_O(g)
            while pending_norms:
                emit_norm(*pending_norms.pop(0))

    nc.compile()
    return nc


def pack_qk(q, k, c):
    qr = q.reshape(B * H, D, N)[c * HPC : (c + 1) * HPC].reshape(NPAIRS, 128, N)
    kr = k.reshape(B * H, D, N)[c * HPC : (c + 1) * HPC].reshape(NPAIRS, 128, N)
    return np.ascontiguousarray(np.stack([qr, kr], axis=1))  # [NPAIRS, 2, 128, N]


def pack_vt(v, c):
    vr = v.reshape(B * H, D, N)[c * HPC : (c + 1) * HPC]  # [8, 64, 2048]
    vr = vr.reshape(NPAIRS, 2, D, NCH, JC)
    # vt[p, j, s, ch, d] = v[pair p, head s, d, ch*128+j]
    vt = np.empty((NPAIRS, JC, 2, NCH, 65), dtype=np.float32)
    vt[..., 0:64] = vr.transpose(0, 4, 1, 3, 2)
    vt[..., 64] = 1.0
    return np.ascontiguousarray(
        vt.reshape(NPAIRS, JC, 2 * NCH * 65).astype(ml_dtypes.bfloat16)
    )


def kernel(q: np.ndarray, k: np.ndarray, v: np.ndarray) -> np.ndarray:
    q = np.asarray(q, dtype=np.float32)
    k = np.asarray(k, dtype=np.float32)
    v = np.asarray(v, dtype=np.float32)
    if "nc" not in _CACHE:
        _CACHE["nc"] = build_bass()
    nc = _CACHE["nc"]

    in_maps = [
        {"qk": pack_qk(q, k, c), "vt": pack_vt(v, c)} for c in range(NCORES)
    ]
    res = run_bass_kernel_spmd(nc, in_maps, core_ids=list(range(NCORES)))
    outs = []
    for c in range(NCORES):
        ot = res.results[c]["out"].reshape(HPC, NIT, 128, 4, 64)
        outs.append(ot.transpose(0, 4, 1, 3, 2).reshape(HPC, D, N))
    out = np.concatenate(outs, axis=0)
    return out.reshape(B, H, D, N).astype(np.float32)


if __name__ == "__main__":
    rng = np.random.default_rng(0)
    q = rng.standard_normal((B, H, D, N), dtype=np.float32)
    k = rng.standard_normal((B, H, D, N), dtype=np.float32)
    v = rng.standard_normal((B, H, D, N), dtype=np.float32)
    out = kernel(q, k, v)
    s = np.einsum("hdi,hdj->hij", q.reshape(-1, D, N)[:2] * SCALE, k.reshape(-1, D, N)[:2])
    p = np.exp(s - s.max(-1, keepdims=True))
    p /= p.sum(-1, keepdims=True)
    ref = np.einsum("hij,hdj->hdi", p, v.reshape(-1, D, N)[:2])
    got = out.reshape(-1, D, N)[:2]
    print("rel err (2 heads):", np.linalg.norm(got - ref) / np.linalg.norm(ref))


# revision 20
# speedup vs baseline: 1.1679x; 1.1679x over previous
"""Trainium2 Bass kernel for batched multi-head attention.

Problem: q, k, v: [B=4, H=16, D=64, N=2048] fp32, layout (b, h, d, n).
    sim  = einsum('bhdi,bhdj->bhij', q * D**-0.5, k)
    attn = softmax(sim, axis=-1)
    out  = einsum('bhij,bhdj->bhdi', attn, v)

Sharding: the 64 (b, h) pairs split across 8 NeuronCores, 8 heads per core
(4 pairs of 2 heads stacked on 128 partitions). No collectives.

Per-core algorithm (flash-style, no max subtraction; logits are O(1)):
  stream of 512 units (pair, sub-head, i-tile of 512, j-chunk of 128),
  grouped 2 units per 2-bank PSUM slot:
    S^T[j, i] = K_chunk^T Q_tile          (fp32r matmul, 512 rows)
    expS      = exp(S * scale) -> bf16    (split across THREE engines, see below)
    O^T[i, d|sum] += expS_chunk^T [V^T|1] (bf16 matmuls, transposed-PV: output
                                           free dim is 65, 4 matmuls per unit,
                                           accumulated over the 16 j-chunks)
    out^T[i, d] = O^T[d] / O^T[sum]       (Pool engine divide)
  V^T arrives pre-transposed from the host with a fused ones column; the
  output is written as out^T [h, n, d] tiles and un-transposed on the host.

Engine split of the 256 exp groups (the softmax exp is the scalar-engine
roofline; PE is the overall roofline at ~170us/core):
  ACT:  exact exp activation, bf16 out                     (~140 groups)
  DVE:  two-seed Schraudolph: z1 = int16(S*A + B2);        (~72 groups)
        z2 = z1 + 64; ex = bf16(z1) + bf16(z2)
        (the +64 int16 offset is a half-period shift = *sqrt(2); summing the
        two seeds phase-averages the Schraudolph sawtooth; the 1+sqrt(2)
        scale is folded into B2 and cancels in softmax normalization)
  Pool: single-seed Schraudolph z1 = int16(S*A + B1), bitcast bf16
        (~44 groups; Pool also runs the normalization divides)
"""

import numpy as np
import ml_dtypes

import concourse.bass as bass
import concourse.mybir as mybir
import concourse.tile as tile
from concourse import bacc
from concourse.bass_utils import run_bass_kernel_spmd

B, H, D, N = 4, 16, 64, 2048
NCORES = 8
HPC = (B * H) // NCORES  # heads per core = 8
NPAIRS = HPC // 2        # head pairs per core = 4
ITILE = 512              # query columns per i-tile
NIT = N // ITILE         # 4 i-tiles
JC = 128                 # key chunk (contraction partition dim)
NCH = N // JC            # 16 chunks
SCALE = float(D) ** -0.5
GW = 2                   # units per group (2-bank PSUM slots)
NGROUPS = (NPAIRS * 2 * NIT * NCH) // GW  # 256

# Schraudolph constants (bf16/int16 bit trick), tuned numerically against the
# exact softmax: z = S * A + B, truncating fp32->int16 convert.
LOG2E = 1.4426950408889634
EXP_A = float(SCALE * LOG2E * 128.0)
EXP_B2 = float(127 * 128 - np.log2(1.0 + np.sqrt(2.0)) * 128.0 - 7.0)  # two-seed
EXP_B1 = float(127 * 128 - 7.0)                                        # one-seed

# exp engine assignment: counts over the 256 groups
N_DVE1 = 124  # single-seed Schraudolph on DVE
N_DVE2 = 0    # two-seed Schraudolph on DVE (slower overall: serial DVE chain)
RAMP_ACT = 4  # first groups forced to ACT (warm start)
TAIL_ACT = 0  # tail alternates engines (parallel X drain)
O_LAG = 8     # groups between exp(g) and its PV consumption O(g)
N_SPLIT_TAIL = 0  # split-X tail measured slower; disabled
USE_RINGS = False  # multi-ring pair-0 DMA measured slower; disabled
NORM_COPY = "act"  # which engine copies the PSUM accumulator to SBUF

F32 = mybir.dt.float32
F32R = mybir.dt.float32r
BF16 = mybir.dt.bfloat16
I16 = mybir.dt.int16

_CACHE = {}


def assign_engines():
    """Per-group engine: 'A' (ACT exact exp), 'D' (DVE 1-seed Schraudolph),
    'E' (DVE 2-seed Schraudolph)."""
    eng = ["A"] * NGROUPS
    free = list(range(RAMP_ACT, NGROUPS - TAIL_ACT - N_SPLIT_TAIL))
    nfree = len(free)
    acc_d = acc_e = 0.0
    for g in free:
        acc_d += N_DVE1 / nfree
        acc_e += N_DVE2 / nfree
        if acc_e >= 1.0:
            eng[g] = "E"
            acc_e -= 1.0
        elif acc_d >= 1.0:
            eng[g] = "D"
            acc_d -= 1.0
    for i, g in enumerate(range(NGROUPS - TAIL_ACT - N_SPLIT_TAIL, NGROUPS - TAIL_ACT)):
        eng[g] = "S"
    return eng


def build_bass():
    nc = bacc.Bacc("TRN2", target_bir_lowering=False)
    qk_h = nc.dram_tensor("qk", [NPAIRS, 2, 128, N], F32, kind="ExternalInput")
    vt_h = nc.dram_tensor("vt", [NPAIRS, 128, 2 * NCH * 65], BF16, kind="ExternalInput")
    o_h = nc.dram_tensor("out", [HPC, NIT, 128, 4 * 64], F32, kind="ExternalOutput")

    qk_d = qk_h[:, :, :, :].rearrange("p t a n -> p a t n")  # [NPAIRS, 128, 2, N]

    eng = assign_engines()
    units = [
        (p, sub, it, c)
        for p in range(NPAIRS)
        for sub in range(2)
        for it in range(NIT)
        for c in range(NCH)
    ]
    groups = [units[i : i + GW] for i in range(0, len(units), GW)]
    assert len(groups) == NGROUPS

    with tile.TileContext(nc) as tc:
        with (
            tc.tile_pool(name="consts", bufs=1) as consts,
            tc.tile_pool(name="pairs", bufs=2) as pairs,
            tc.tile_pool(name="vtp", bufs=2) as vtp,
            tc.tile_pool(name="spsum", bufs=3, space="PSUM") as spsum,
            tc.tile_pool(name="opsum", bufs=2, space="PSUM") as opsum,
            tc.tile_pool(name="expp", bufs=10) as expp,
            tc.tile_pool(name="zp", bufs=10) as zp,
            tc.tile_pool(name="outp", bufs=4) as outp,
        ):
            # dummy exp so the ACT table load lands at t=0 on real hw
            dum = consts.tile([1, 8], F32, tag="dum")
            nc.vector.memset(dum, 0.0)
            nc.scalar.activation(out=dum, in_=dum, func=mybir.ActivationFunctionType.Exp)
            # PE warm-up at t=0: starts the p-state ramp clock early
            wup = consts.tile([128, 64], BF16, tag="wup")
            nc.vector.memset(wup, 0.0)
            wps = opsum.tile([128, 260], F32, tag="o", name="warm")
            nc.tensor.matmul(out=wps[0:64, 0:64], lhsT=wup, rhs=wup, start=True, stop=True)

            pair_ctx: list[dict | None] = [None] * NPAIRS

            def emit_pair_dma(p):
                qk = pairs.tile([128, 2, N], F32R, tag="qk", name=f"qk_{p}")
                src = qk_d[p].bitcast(F32R)
                vt = vtp.tile([128, 2, NCH, 65], BF16, tag="vt", name=f"vt_{p}")
                if p == 0:
                    # ramp: q, k, vt can ride separate DGE rings in parallel
                    # (ACT/SP/Pool), peeled in dependency order
                    q_eng = nc.gpsimd if USE_RINGS else nc.sync
                    v_eng = nc.gpsimd if USE_RINGS else nc.sync
                    q_eng.dma_start(out=qk[:, 0, 0:128], in_=src[:, 0, 0:128])
                    nc.sync.dma_start(out=qk[:, 1, 0:256], in_=src[:, 1, 0:256])
                    q_eng.dma_start(out=qk[:, 0, 128:ITILE], in_=src[:, 0, 128:ITILE])
                    nc.sync.dma_start(out=qk[:, 1, 256:768], in_=src[:, 1, 256:768])
                    v_eng.dma_start(
                        out=vt.rearrange("a b c d -> a (b c d)"), in_=vt_h[p]
                    )
                    nc.sync.dma_start(out=qk[:, 1, 768:1536], in_=src[:, 1, 768:1536])
                    nc.sync.dma_start(out=qk[:, 1, 1536:N], in_=src[:, 1, 1536:N])
                    q_eng.dma_start(out=qk[:, 0, ITILE:1024], in_=src[:, 0, ITILE:1024])
                    q_eng.dma_start(out=qk[:, 0, 1024:N], in_=src[:, 0, 1024:N])
                else:
                    nc.sync.dma_start(out=qk[:, 0:1, :], in_=src[:, 0:1, :])
                    nc.sync.dma_start(out=qk[:, 1:2, :], in_=src[:, 1:2, :])
                    nc.sync.dma_start(
                        out=vt.rearrange("a b c d -> a (b c d)"), in_=vt_h[p]
                    )
                pair_ctx[p] = {"qk": qk, "vt": vt}

            slots: dict[int, bass.AP] = {}
            exs: dict[int, bass.AP] = {}
            o_accs: dict[tuple, bass.AP] = {}
            pending_norms: list[tuple] = []

            def emit_S(g):
                slot = spsum.tile([128, GW * ITILE], F32, tag="s", name=f"s_{g}")
                for kk, (p, sub, it, c) in enumerate(groups[g]):
                    if sub == 0 and it == 0 and c == 0:
                        if p == 0:
                            emit_pair_dma(0)
                        if p + 1 < NPAIRS:
                            emit_pair_dma(p + 1)
                    qk = pair_ctx[p]["qk"]
                    hb = sub * D
                    if g == 0 and kk == 0:
                        # ramp: sub-matmuls per 128 q columns so the PE starts
                        # as soon as the first 64KB of q/k have landed (the
                        # first sub's start=True zeroes the whole bank)
                        for j in range(4):
                            nc.tensor.matmul(
                                out=slot[:, j * 128 : (j + 1) * 128],
                                lhsT=qk[hb : hb + D, 1, c * JC : (c + 1) * JC],
                                rhs=qk[hb : hb + D, 0, j * 128 : (j + 1) * 128],
                                start=(j == 0),
                                stop=True,
                            )
                    else:
                        nc.tensor.matmul(
                            out=slot[:, kk * ITILE : (kk + 1) * ITILE],
                            lhsT=qk[hb : hb + D, 1, c * JC : (c + 1) * JC],
                            rhs=qk[hb : hb + D, 0, it * ITILE : (it + 1) * ITILE],
                            start=True,
                            stop=True,
                        )
                slots[g] = slot

            def emit_X(g):
                slot = slots.pop(g)
                w = GW * ITILE
                e = eng[g]
                if e == "A":
                    ex = expp.tile([128, w], BF16, tag="exA", name=f"xa_{g}")
                    nc.scalar.activation(
                        out=ex, in_=slot, func=mybir.ActivationFunctionType.Exp,
                        scale=SCALE,
                    )
                elif e == "S":
                    # split-X: ACT does the exact exp on the first half while
                    # DVE Schraudolphs the second half, in parallel
                    exi = zp.tile([128, w], I16, tag="exS", name=f"xs_{g}")
                    nc.scalar.activation(
                        out=exi[:, 0 : w // 2].bitcast(BF16),
                        in_=slot[:, 0 : w // 2],
                        func=mybir.ActivationFunctionType.Exp,
                        scale=SCALE,
                    )
                    nc.vector.tensor_scalar(
                        out=exi[:, w // 2 : w], in0=slot[:, w // 2 : w],
                        scalar1=EXP_A, scalar2=EXP_B1,
                        op0=mybir.AluOpType.mult, op1=mybir.AluOpType.add,
                    )
                    ex = exi.bitcast(BF16)
                elif e == "D":  # DVE single-seed Schraudolph
                    z1 = zp.tile([128, w], I16, tag="z1", name=f"z1_{g}")
                    nc.vector.tensor_scalar(
                        out=z1, in0=slot, scalar1=EXP_A, scalar2=EXP_B1,
                        op0=mybir.AluOpType.mult, op1=mybir.AluOpType.add,
                    )
                    ex = z1.bitcast(BF16)
                else:  # DVE two-seed Schraudolph (phase-averaged)
                    z1 = zp.tile([128, w], I16, tag="z1", name=f"z1_{g}")
                    nc.vector.tensor_scalar(
                        out=z1, in0=slot, scalar1=EXP_A, scalar2=EXP_B2,
                        op0=mybir.AluOpType.mult, op1=mybir.AluOpType.add,
                    )
                    z2 = zp.tile([128, w], I16, tag="z2", name=f"z2_{g}")
                    nc.vector.tensor_scalar(
                        out=z2, in0=z1, scalar1=64, scalar2=None,
                        op0=mybir.AluOpType.add,
                    )
                    ex = expp.tile([128, w], BF16, tag="exE", name=f"xe_{g}")
                    nc.vector.tensor_tensor(
                        out=ex, in0=z1.bitcast(BF16), in1=z2.bitcast(BF16),
                        op=mybir.AluOpType.add,
                    )
                exs[g] = ex

            def emit_O(g):
                ex = exs.pop(g)
                for kk, (p, sub, it, c) in enumerate(groups[g]):
                    vt = pair_ctx[p]["vt"]
                    if c == 0:
                        o_accs[(p, sub, it)] = opsum.tile(
                            [128, 4 * 65], F32, tag="o", name=f"oa_{g}_{kk}"
                        )
                    acc = o_accs[(p, sub, it)]
                    for si in range(4):
                        # start=True zeroes the WHOLE psum bank, so only the
                        # very first matmul of this accumulator may set it;
                        # the other si chains accumulate onto the zeroed bank.
                        nc.tensor.matmul(
                            out=acc[:, si * 65 : (si + 1) * 65],
                            lhsT=ex[:, kk * ITILE + si * 128 : kk * ITILE + (si + 1) * 128],
                            rhs=vt[:, sub, c, :],
                            start=(c == 0 and si == 0),
                            stop=(c == NCH - 1),
                        )
                    if c == NCH - 1:
                        pending_norms.append((p, sub, it))

            def emit_norm(p, sub, it):
                # Pool (gpsimd) cannot read PSUM: DVE copies the accumulator
                # to SBUF in one op, Pool does the four divides from there.
                acc = o_accs.pop((p, sub, it))
                acc_sb = outp.tile([128, 4 * 65], F32, tag="accsb", name=f"ac_{p}_{sub}_{it}")
                use_act = NORM_COPY == "act" or (
                    NORM_COPY == "alt" and (p * 2 + sub + it) % 2 == 0
                )
                if use_act:
                    nc.scalar.activation(
                        out=acc_sb, in_=acc, func=mybir.ActivationFunctionType.Copy
                    )
                else:
                    nc.vector.tensor_copy(out=acc_sb, in_=acc)
                out_sb = outp.tile([128, 4, 64], F32, tag="osb", name=f"ot_{p}_{sub}_{it}")
                for si in range(4):
                    nc.gpsimd.normalize_recip(
                        out_sb[:, si, :],
                        acc_sb[:, si * 65 : si * 65 + 64],
                        acc_sb[:, si * 65 + 64 : si * 65 + 65],
                    )
                nc.sync.dma_start(
                    out=o_h[2 * p + sub, it],
                    in_=out_sb.rearrange("a b c -> a (b c)"),
                )

            def flush_norms(g):
                while pending_norms:
                    emit_norm(*pending_norms.pop(0))

            # software-pipelined emission: PE stream ... S(g+1) O(g-1) S(g+2) ...
            emit_S(0)
            for g in range(NGROUPS):
                emit_X(g)
                if g + 1 < NGROUPS:
                    emit_S(g + 1)
                if g >= O_LAG:
                    emit_O(g - O_LAG)
                flush_norms(g)
            for g in range(NGROUPS - O_LAG, NGROUPS):
                emit_O(g)
            while pending_norms:
                emit_norm(*pending_norms.pop(0))

    nc.compile()
    return nc


def pack_qk(q, k, c):
    qr = q.reshape(B * H, D, N)[c * HPC : (c + 1) * HPC].reshape(NPAIRS, 128, N)
    kr = k.reshape(B * H, D, N)[c * HPC : (c + 1) * HPC].reshape(NPAIRS, 128, N)
    return np.ascontiguousarray(np.stack([qr, kr], axis=1))  # [NPAIRS, 2, 128, N]


def pack_vt(v, c):
    vr = v.reshape(B * H, D, N)[c * HPC : (c + 1) * HPC]  # [8, 64, 2048]
    vr = vr.reshape(NPAIRS, 2, D, NCH, JC)
    # vt[p, j, s, ch, d] = v[pair p, head s, d, ch*128+j]
    vt = np.empty((NPAIRS, JC, 2, NCH, 65), dtype=np.float32)
    vt[..., 0:64] = vr.transpose(0, 4, 1, 3, 2)
    vt[..., 64] = 1.0
    return np.ascontiguousarray(
        vt.reshape(NPAIRS, JC, 2 * NCH * 65).astype(ml_dtypes.bfloat16)
    )


def kernel(q: np.ndarray, k: np.ndarray, v: np.ndarray) -> np.ndarray:
    q = np.asarray(q, dtype=np.float32)
    k = np.asarray(k, dtype=np.float32)
    v = np.asarray(v, dtype=np.float32)
    if "nc" not in _CACHE:
        _CACHE["nc"] = build_bass()
    nc = _CACHE["nc"]

    in_maps = [
        {"qk": pack_qk(q, k, c), "vt": pack_vt(v, c)} for c in range(NCORES)
    ]
    res = run_bass_kernel_spmd(nc, in_maps, core_ids=list(range(NCORES)))
    outs = []
    for c in range(NCORES):
        ot = res.results[c]["out"].reshape(HPC, NIT, 128, 4, 64)
        outs.append(ot.transpose(0, 4, 1, 3, 2).reshape(HPC, D, N))
    out = np.concatenate(outs, axis=0)
    return out.reshape(B, H, D, N).astype(np.float32)


if __name__ == "__main__":
    rng = np.random.default_rng(0)
    q = rng.standard_normal((B, H, D, N), dtype=np.float32)
    k = rng.standard_normal((B, H, D, N), dtype=np.float32)
    v = rng.standard_normal((B, H, D, N), dtype=np.float32)
    out = kernel(q, k, v)
    s = np.einsum("hdi,hdj->hij", q.reshape(-1, D, N)[:2] * SCALE, k.reshape(-1, D, N)[:2])
    p = np.exp(s - s.max(-1, keepdims=True))
    p /= p.sum(-1, keepdims=True)
    ref = np.einsum("hij,hdj->hdi", p, v.reshape(-1, D, N)[:2])
    got = out.reshape(-1, D, N)[:2]
    print("rel err (2 heads):", np.linalg.norm(got - ref) / np.linalg.norm(ref))
